# revision 11
# baseline (speedup 1.0000x reference)
"""Trainium2 Bass kernel for the AnomalyBlock problem.

Strategy: data-parallel over batch B=8 (one batch element per NeuronCore).
Each core runs the full attention block for its batch element:
  q/k/v/o projections (float32r matmuls, biases folded in as K=1 rank-1
  matmuls against a ones-row), scores in BOTH orientations ([l,s] for the
  normalized attention output, [s,l] for the A@V contraction since the PE
  contracts over the partition dim), softmax via ACT exp with accum_out
  (row sums for free) + DVE per-partition normalize.
prior_attn is batch-broadcast, so only [H,L,L] is unique; its rows are
sharded across the 8 cores (core c computes rows c*128..c*128+127 for all
heads) and the batch dim is broadcast on the host at gather time.
"""
import sys
import numpy as np

sys.path.insert(0, "/opt/trn_rl_repo")

B, L, D, H, DK = 8, 1024, 256, 8, 32
N_CORES = 8
SCALE = 1.0 / np.sqrt(DK)

_CACHE = {}


def _build(repeat=1):
    import concourse.bacc as bacc
    import concourse.mybir as mybir
    import concourse.tile as tile

    F32 = mybir.dt.float32
    F32R = mybir.dt.float32r
    AF = mybir.ActivationFunctionType
    ALU = mybir.AluOpType

    nc = bacc.Bacc("TRN2", target_bir_lowering=False, debug=False,
                   num_devices=N_CORES)

    x_d = nc.dram_tensor("x", [L, D], F32, kind="ExternalInput")
    w_d = {k: nc.dram_tensor(f"W{k}", [D, D], F32, kind="ExternalInput")
           for k in "qkvo"}
    b_d = {k: nc.dram_tensor(f"b{k}", [1, D], F32, kind="ExternalInput")
           for k in "qkvo"}
    sig_d = nc.dram_tensor("sig", [1, H], F32, kind="ExternalInput")
    dist2n_d = nc.dram_tensor("dist2n", [128, L], F32, kind="ExternalInput")
    eye_d = nc.dram_tensor("eye", [128, 128], F32, kind="ExternalInput")
    ones_d = nc.dram_tensor("ones", [1, L], F32, kind="ExternalInput")
    bsel_d = nc.dram_tensor("bsel", [8, 2, 128], F32, kind="ExternalInput")

    attn_d = nc.dram_tensor("attn", [H, L, L], F32, kind="ExternalOutput")
    prior_d = nc.dram_tensor("prior", [H, 128, L], F32, kind="ExternalOutput")
    out_d = nc.dram_tensor("out", [L, D], F32, kind="ExternalOutput")

    with tile.TileContext(nc) as tc:
        with (
            tc.tile_pool(name="const", bufs=1) as cpool,
            tc.tile_pool(name="attnp", bufs=8) as pattn,
            tc.tile_pool(name="eTp", bufs=10) as peT,
            tc.tile_pool(name="tinyp", bufs=8) as ptiny,
            tc.tile_pool(name="mm2", bufs=3, space="PSUM") as pmm2,
            tc.tile_pool(name="avp", bufs=1, space="PSUM") as pav,
            tc.tile_pool(name="smallp", bufs=1, space="PSUM") as psmall,
        ):
            # ---- Phase 0: input DMAs ----
            x_sb = cpool.tile([128, 8, D], F32)
            nc.sync.dma_start(x_sb[:], x_d.rearrange("(a p) d -> p a d", p=128))
            w_sb = {}
            for k in "qkvo":
                w_sb[k] = cpool.tile([128, 2, D], F32R, tag=f"W{k}", name=f"W{k}_sb")
                nc.gpsimd.dma_start(
                    w_sb[k][:], w_d[k].rearrange("(k p) n -> p k n", p=128))
            b_sb = {}
            for k in "qkvo":
                b_sb[k] = cpool.tile([1, D], F32R, tag=f"b{k}", name=f"b{k}_sb")
                nc.gpsimd.dma_start(b_sb[k][:], b_d[k][:])
            sig_sb = cpool.tile([1, H], F32)
            nc.sync.dma_start(sig_sb[:], sig_d[:])
            dist2n_sb = cpool.tile([128, L], F32)
            nc.sync.dma_start(dist2n_sb[:], dist2n_d[:])
            eye_sb = cpool.tile([128, 128], F32)
            nc.sync.dma_start(eye_sb[:], eye_d[:])
            ones_f = cpool.tile([1, L], F32)
            nc.sync.dma_start(ones_f[:], ones_d[:])
            ones_r = cpool.tile([1, L], F32R)
            nc.gpsimd.dma_start(ones_r[:], ones_d[:])
            bsel_sb = cpool.tile([8, 2, 128], F32R)
            nc.gpsimd.dma_start(bsel_sb[:], bsel_d[:])

            # ---- Phases 1-6, optionally repeated for benchmarking ----
            for _rep in range(repeat):
                _emit_compute(nc, tc, cpool, pattn, peT, ptiny, pmm2, pav,
                              psmall, F32, F32R, AF, ALU, x_sb, w_sb, b_sb,
                              sig_sb, dist2n_sb, eye_sb, ones_f, ones_r,
                              bsel_sb, attn_d, prior_d, out_d, _rep)

    nc.compile()
    return nc


def _emit_compute(nc, tc, cpool, pattn, peT, ptiny, pmm2, pav, psmall,
                  F32, F32R, AF, ALU, x_sb, w_sb, b_sb, sig_sb, dist2n_sb,
                  eye_sb, ones_f, ones_r, bsel_sb, attn_d, prior_d, out_d,
                  _rep):
    if True:
        if True:
            # ---- Phase 1: xT = x transposed, [d, l] layout, f32r ----
            xT = [cpool.tile([128, L], F32R, tag=f"xT{dc}", name=f"xT{dc}_{_rep}") for dc in range(2)]
            for li in range(8):
                for dc in range(2):
                    ps = psmall.tile([128, 128], F32, tag="small", name="ps")
                    nc.tensor.transpose(
                        ps[:], x_sb[:, li, dc * 128:(dc + 1) * 128], eye_sb[:])
                    nc.vector.tensor_copy(
                        xT[dc][:, li * 128:(li + 1) * 128], ps[:])

            # ---- Phase 2: projections ----
            # QT/KT: [dout, l] layout (head-major on partitions), f32r
            QT = [cpool.tile([128, L], F32R, tag=f"QT{dc}", name=f"QT{dc}_{_rep}") for dc in range(2)]
            KT = [cpool.tile([128, L], F32R, tag=f"KT{dc}", name=f"KT{dc}_{_rep}") for dc in range(2)]
            for name, wk, bk, dst in (("q", "q", "q", QT), ("k", "k", "k", KT)):
                for dc in range(2):
                    for lh in range(2):
                        ps = psmall.tile([128, 512], F32, tag="small", name="ps")
                        for kc in range(2):
                            nc.tensor.matmul(
                                ps[:],
                                w_sb[wk][:, kc, dc * 128:(dc + 1) * 128],
                                xT[kc][:, lh * 512:(lh + 1) * 512],
                                start=(kc == 0), stop=False)
                        nc.tensor.matmul(
                            ps[:],
                            b_sb[bk][0:1, dc * 128:(dc + 1) * 128],
                            ones_r[0:1, lh * 512:(lh + 1) * 512],
                            start=False, stop=True)
                        nc.vector.tensor_copy(
                            dst[dc][:, lh * 512:(lh + 1) * 512], ps[:])
            # V: natural [s, dout] layout, f32r
            V_sb = cpool.tile([128, 8, D], F32R)
            for si in range(8):
                ps = psmall.tile([128, D], F32, tag="small", name="ps")
                for kc in range(2):
                    nc.tensor.matmul(
                        ps[:], xT[kc][:, si * 128:(si + 1) * 128],
                        w_sb["v"][:, kc, :], start=(kc == 0), stop=False)
                nc.tensor.matmul(ps[:], ones_r[0:1, 0:128], b_sb["v"][0:1, :],
                                 start=False, stop=True)
                nc.vector.tensor_copy(V_sb[:, si, :], ps[:])

            # ---- Phase 3: prior ----
            t0 = cpool.tile([1, H], F32, tag="t0")
            nc.scalar.activation(t0[:], sig_sb[:], AF.Abs)
            nc.vector.tensor_scalar_add(t0[:], t0[:], 1e-6)
            nc.vector.tensor_tensor(t0[:], t0[:], t0[:], op=ALU.mult)
            nc.vector.tensor_scalar_mul(t0[:], t0[:], 2.0)
            inv2s = cpool.tile([1, H], F32, tag="inv2s")
            nc.vector.reciprocal(inv2s[:], t0[:])
            ps_sc = psmall.tile([128, H], F32, tag="small")
            nc.tensor.matmul(ps_sc[:], ones_f[0:1, 0:128], inv2s[:],
                             start=True, stop=True)
            scales = cpool.tile([128, H], F32)
            nc.vector.tensor_copy(scales[:], ps_sc[:])
            prsums = cpool.tile([128, H], F32)
            for h in range(H):
                pr = pattn.tile([128, L], F32, tag="attn", name="pr")
                nc.scalar.activation(pr[:], dist2n_sb[:], AF.Exp,
                                     scale=scales[:, h:h + 1],
                                     accum_out=prsums[:, h:h + 1])
                ssum = ptiny.tile([128, 1], F32, tag="tiny", name="ssum")
                nc.vector.tensor_scalar_add(ssum[:], prsums[:, h:h + 1], 1e-8)
                nc.vector.reciprocal(ssum[:], ssum[:])
                nc.vector.tensor_scalar_mul(pr[:], pr[:], ssum[:])
                nc.sync.dma_start(prior_d[h], pr[:])

            # ---- Phase 4: scores [l,s] -> exp -> normalize -> DMA ----
            sums = cpool.tile([128, 8, H], F32)
            for li in range(8):
                for h in range(H):
                    dc, hp = h // 4, h % 4
                    pss = pmm2.tile([128, L], F32, tag="mm2", name="pss")
                    for sh in range(2):
                        nc.tensor.matmul(
                            pss[:, sh * 512:(sh + 1) * 512],
                            QT[dc][hp * 32:(hp + 1) * 32,
                                   li * 128:(li + 1) * 128],
                            KT[dc][hp * 32:(hp + 1) * 32,
                                   sh * 512:(sh + 1) * 512],
                            start=True, stop=True,
                            tile_position=(hp * 32, 0))
                    e = pattn.tile([128, L], F32, tag="attn", name="e")
                    nc.scalar.activation(e[:], pss[:], AF.Exp, scale=SCALE,
                                         accum_out=sums[:, li, h:h + 1])
                    inv1 = ptiny.tile([128, 1], F32, tag="tiny", name="inv1")
                    nc.vector.reciprocal(inv1[:], sums[:, li, h:h + 1])
                    nc.vector.tensor_scalar_mul(e[:], e[:], inv1[:])
                    nc.sync.dma_start(
                        attn_d[h, li * 128:(li + 1) * 128, :], e[:])

            # invSrows: [8, 1024] = 1/S[h,l] (for normalizing A@V later)
            ps_sr = pmm2.tile([8, L], F32, tag="mm2")
            for li in range(8):
                nc.tensor.transpose(ps_sr[0:8, li * 128:(li + 1) * 128],
                                    sums[:, li, :], eye_sb[:])
            invSrows = cpool.tile([8, L], F32)
            nc.vector.reciprocal(invSrows[:], ps_sr[0:8, :])
            invSrows_r = cpool.tile([8, L], F32R)
            nc.vector.tensor_copy(invSrows_r[:], invSrows[:])

            # ---- Phase 5: scoresT -> expT -> A@V accumulate; normalize ----
            wUT = [cpool.tile([128, L], F32R, tag=f"wUT{g}", name=f"wUT{g}_{_rep}")
                   for g in range(2)]
            for g in range(2):
                # broadcast 1/S[h,l] over each head's 32 partitions via a
                # selection-matrix matmul (K=8)
                psbc = pmm2.tile([128, L], F32, tag="mm2", name="psbc")
                for lh in range(2):
                    nc.tensor.matmul(
                        psbc[:, lh * 512:(lh + 1) * 512],
                        bsel_sb[:, g, :],
                        invSrows_r[:, lh * 512:(lh + 1) * 512],
                        start=True, stop=True)
                invSbc = cpool.tile([128, L], F32, tag=f"invSbc{g}",
                                    name=f"invSbc{g}_{_rep}")
                nc.vector.tensor_copy(invSbc[:], psbc[:])
                for hp in range(4):
                    h = g * 4 + hp
                    eTs = []
                    for si in range(8):
                        pst = pmm2.tile([128, L], F32, tag="mm2", name="pst")
                        for lh in range(2):
                            nc.tensor.matmul(
                                pst[:, lh * 512:(lh + 1) * 512],
                                KT[g][hp * 32:(hp + 1) * 32,
                                      si * 128:(si + 1) * 128],
                                QT[g][hp * 32:(hp + 1) * 32,
                                      lh * 512:(lh + 1) * 512],
                                start=True, stop=True,
                                tile_position=(hp * 32, 0))
                        eT = peT.tile([128, L], F32R, tag="eT", name="eT")
                        nc.scalar.activation(eT[:], pst[:], AF.Exp, scale=SCALE)
                        eTs.append(eT)
                    for lh in range(2):
                        pvh = pav.tile([32, 512], F32, tag="av", name="pvh")
                        for si in range(8):
                            nc.tensor.matmul(
                                pvh[:],
                                V_sb[:, si, h * 32:(h + 1) * 32],
                                eTs[si][:, lh * 512:(lh + 1) * 512],
                                start=(si == 0), stop=(si == 7))
                        nc.vector.tensor_tensor(
                            wUT[g][hp * 32:(hp + 1) * 32,
                                   lh * 512:(lh + 1) * 512],
                            pvh[:],
                            invSbc[hp * 32:(hp + 1) * 32,
                                   lh * 512:(lh + 1) * 512],
                            op=ALU.mult)

            # ---- Phase 6: out = weighted @ Wo + bo, transpose, DMA ----
            outT = [cpool.tile([128, L], F32, tag=f"outT{go}", name=f"outT{go}_{_rep}")
                    for go in range(2)]
            for go in range(2):
                for lh in range(2):
                    ps = psmall.tile([128, 512], F32, tag="small", name="ps")
                    for gi in range(2):
                        nc.tensor.matmul(
                            ps[:], w_sb["o"][:, gi, go * 128:(go + 1) * 128],
                            wUT[gi][:, lh * 512:(lh + 1) * 512],
                            start=(gi == 0), stop=False)
                    nc.tensor.matmul(
                        ps[:], b_sb["o"][0:1, go * 128:(go + 1) * 128],
                        ones_r[0:1, lh * 512:(lh + 1) * 512],
                        start=False, stop=True)
                    nc.vector.tensor_copy(outT[go][:, lh * 512:(lh + 1) * 512], ps[:])
            for li in range(8):
                pso = psmall.tile([128, D], F32, tag="small", name="pso")
                for go in range(2):
                    nc.tensor.transpose(
                        pso[:, go * 128:(go + 1) * 128],
                        outT[go][:, li * 128:(li + 1) * 128], eye_sb[:])
                onat = pattn.tile([128, D], F32, tag="onat", name="onat")
                nc.vector.tensor_copy(onat[:], pso[:])
                nc.sync.dma_start(out_d[li * 128:(li + 1) * 128, :], onat[:])


def _get_nc():
    if "nc" not in _CACHE:
        _CACHE["nc"] = _build()
    return _CACHE["nc"]


def make_in_maps(inputs):
    x = np.asarray(inputs["x"], dtype=np.float32)
    ws = {k: np.ascontiguousarray(np.asarray(inputs[f"W{k}"], np.float32))
          for k in "qkvo"}
    bs = {k: np.ascontiguousarray(
        np.asarray(inputs[f"b{k}"], np.float32).reshape(1, D))
        for k in "qkvo"}
    sig = np.asarray(inputs["prior_sigma"], np.float32).reshape(1, H)
    eye = np.eye(128, dtype=np.float32)
    ones = np.ones((1, L), dtype=np.float32)
    bsel = np.zeros((8, 2, 128), dtype=np.float32)
    for g in range(2):
        for h in range(8):
            for p in range(128):
                if h == g * 4 + p // 32:
                    bsel[h, g, p] = 1.0
    pos = np.arange(L, dtype=np.float64)

    in_maps = []
    for c in range(N_CORES):
        rows = pos[c * 128:(c + 1) * 128]
        dist2n = (-((rows[:, None] - pos[None, :]) ** 2)).astype(np.float32)
        m = {"x": np.ascontiguousarray(x[c]), "sig": sig, "dist2n": dist2n,
             "eye": eye, "ones": ones, "bsel": bsel}
        for k in "qkvo":
            m[f"W{k}"] = ws[k]
            m[f"b{k}"] = bs[k]
        in_maps.append(m)
    return in_maps


def kernel(**inputs):
    from concourse.bass_utils import run_bass_kernel_spmd

    nc = _get_nc()
    in_maps = make_in_maps(inputs)
    res = run_bass_kernel_spmd(nc, in_maps, list(range(N_CORES)))
    _CACHE["last_results"] = res

    out = np.stack([res.results[c]["out"] for c in range(N_CORES)])
    series = np.stack([res.results[c]["attn"] for c in range(N_CORES)])
    prior_h = np.concatenate(
        [res.results[c]["prior"] for c in range(N_CORES)], axis=1)
    prior = np.broadcast_to(prior_h[None], (B, H, L, L))
    return (out, series, prior)


# revision 29
# speedup vs baseline: 1.4050x; 1.4050x over previous
"""Trainium2 Bass kernel for the AnomalyBlock problem.

Strategy: data-parallel over batch B=8 (one batch element per NeuronCore).
Each core runs the full attention block for its batch element:
  - q/k/v projections as float32r matmuls against a host-pre-transposed
    xT; biases folded in as K=1 rank-1 matmuls against a ones-row.
  - scores computed in BOTH orientations: [l,s] for the normalized
    attention-map output (ACT exp with accum_out giving row sums for
    free, DVE per-partition normalize), and [s,l] for the A@V
    contraction (the PE contracts over the partition dim, so the moving
    operand must carry s on partitions).
  - A@V accumulates per head into one PSUM tile; results are copied raw
    to SBUF and the 1/S normalization is applied at the end via a
    selection-matrix matmul broadcast (invS varies per (head, l)).
  - the [l,s] pipeline and the [s,l] pipeline are interleaved per
    l-chunk so DMA (attention-map writes) and ACT (exp) overlap.
prior_attn is batch-broadcast, so only [H,L,L] is unique; its rows are
sharded across the 8 cores (core c computes rows c*128..c*128+127 for
all heads) and the batch dim is broadcast on the host at gather time.
"""
import sys
import numpy as np

sys.path.insert(0, "/opt/trn_rl_repo")

B, L, D, H, DK = 8, 1024, 256, 8, 32
N_CORES = 8
SCALE = 1.0 / np.sqrt(DK)

_CACHE = {}


def _build(repeat=1):
    import concourse.bacc as bacc
    import concourse.mybir as mybir
    import concourse.tile as tile

    F32 = mybir.dt.float32
    F32R = mybir.dt.float32r
    AF = mybir.ActivationFunctionType
    ALU = mybir.AluOpType

    nc = bacc.Bacc("TRN2", target_bir_lowering=False, debug=False,
                   num_devices=N_CORES)

    xT_d = nc.dram_tensor("xT", [2, 128, L], F32, kind="ExternalInput")
    w_d = {k: nc.dram_tensor(f"W{k}", [D, D], F32, kind="ExternalInput")
           for k in "qkvo"}
    b_d = {k: nc.dram_tensor(f"b{k}", [1, D], F32, kind="ExternalInput")
           for k in "qkvo"}
    sig_d = nc.dram_tensor("sig", [1, H], F32, kind="ExternalInput")
    dist2n_d = nc.dram_tensor("dist2n", [128, L], F32, kind="ExternalInput")
    eye_d = nc.dram_tensor("eye", [128, 128], F32, kind="ExternalInput")
    ones_d = nc.dram_tensor("ones", [1, L], F32, kind="ExternalInput")
    bsel_d = nc.dram_tensor("bsel", [8, 2, 128], F32, kind="ExternalInput")

    attn_d = nc.dram_tensor("attn", [H, L, L], F32, kind="ExternalOutput")
    prior_d = nc.dram_tensor("prior", [H, 128, L], F32, kind="ExternalOutput")
    out_d = nc.dram_tensor("out", [L, D], F32, kind="ExternalOutput")

    with tile.TileContext(nc) as tc:
        with (
            tc.tile_pool(name="const", bufs=1) as cpool,
            tc.tile_pool(name="attnp", bufs=2) as pattn,
            tc.tile_pool(name="eTp", bufs=8) as peT,
            tc.tile_pool(name="tinyp", bufs=8) as ptiny,
            tc.tile_pool(name="mm2", bufs=4, space="PSUM") as pmm2,
        ):
            # ---- input DMAs: small ones first so they aren't queued
            # behind the big loads; f32r tensors staged f32 + DVE cast ----
            sig_sb = cpool.tile([1, H], F32)
            nc.sync.dma_start(sig_sb[:], sig_d[:])
            dist2n_sb = cpool.tile([128, L], F32)
            nc.sync.dma_start(dist2n_sb[:], dist2n_d[:])
            eye_sb = cpool.tile([128, 128], F32)
            nc.sync.dma_start(eye_sb[:], eye_d[:])
            ones_f = cpool.tile([1, L], F32)
            nc.sync.dma_start(ones_f[:], ones_d[:])
            ones_r = cpool.tile([1, L], F32R)
            nc.vector.tensor_copy(ones_r[:], ones_f[:])
            xT = [cpool.tile([128, L], F32R, tag=f"xT{dc}", name=f"xT{dc}")
                  for dc in range(2)]
            for dc in range(2):
                stg = pattn.tile([128, L], F32, tag="pr", bufs=2, name="stg")
                nc.sync.dma_start(stg[:], xT_d[dc])
                nc.vector.tensor_copy(xT[dc][:], stg[:])
            w_sb = {}
            for k in "qkvo":
                w_sb[k] = cpool.tile([128, 2, D], F32R, tag=f"W{k}",
                                     name=f"W{k}_sb")
                stgw = pattn.tile([128, 2, D], F32, tag="stgw", bufs=2,
                                  name="stgw")
                nc.sync.dma_start(
                    stgw[:], w_d[k].rearrange("(k p) n -> p k n", p=128))
                nc.vector.tensor_copy(w_sb[k][:], stgw[:])
            b_sb = {}
            for k in "qkvo":
                b_sb[k] = cpool.tile([1, D], F32R, tag=f"b{k}",
                                     name=f"b{k}_sb")
                stgb = pattn.tile([1, D], F32, tag="stgb", bufs=2,
                                  name="stgb")
                nc.sync.dma_start(stgb[:], b_d[k][:])
                nc.vector.tensor_copy(b_sb[k][:], stgb[:])
            bsel_sb = cpool.tile([8, 2, 128], F32R)
            stgs = pattn.tile([8, 2, 128], F32, tag="stgb", bufs=2,
                              name="stgs")
            nc.sync.dma_start(stgs[:], bsel_d[:])
            nc.vector.tensor_copy(bsel_sb[:], stgs[:])

            # ---- sigma -> 1/(2 sigma^2) -> per-partition scales ----
            t0 = cpool.tile([1, H], F32, tag="t0")
            nc.scalar.activation(t0[:], sig_sb[:], AF.Abs)
            nc.vector.tensor_scalar_add(t0[:], t0[:], 1e-6)
            nc.vector.tensor_tensor(t0[:], t0[:], t0[:], op=ALU.mult)
            nc.vector.tensor_scalar_mul(t0[:], t0[:], 2.0)
            inv2s = cpool.tile([1, H], F32, tag="inv2s")
            nc.vector.reciprocal(inv2s[:], t0[:])
            ps_sc = pmm2.tile([128, H], F32, tag="mm2", name="ps_sc")
            nc.tensor.matmul(ps_sc[:], ones_f[0:1, 0:128], inv2s[:],
                             start=True, stop=True)
            scales = cpool.tile([128, H], F32)
            nc.vector.tensor_copy(scales[:], ps_sc[:])

            for _rep in range(repeat):
                _emit_compute(nc, tc, cpool, pattn, peT, ptiny, pmm2,
                              F32, F32R, AF, ALU, xT, w_sb, b_sb, scales,
                              dist2n_sb, eye_sb, ones_f, ones_r, bsel_sb,
                              attn_d, prior_d, out_d, _rep)

    nc.compile()
    return nc


def _emit_compute(nc, tc, cpool, pattn, peT, ptiny, pmm2,
                  F32, F32R, AF, ALU, xT, w_sb, b_sb, scales, dist2n_sb,
                  eye_sb, ones_f, ones_r, bsel_sb, attn_d, prior_d, out_d,
                  _rep):
    # ---- prior tiles: exp(dist2n * scale_h), row-normalized ----
    prsums = cpool.tile([128, H], F32, tag="prsums", name=f"prsums_{_rep}")
    for h in range(H):
        pr = pattn.tile([128, L], F32, tag="pr", bufs=2, name="pr")
        nc.scalar.activation(pr[:], dist2n_sb[:], AF.Exp,
                             scale=scales[:, h:h + 1],
                             accum_out=prsums[:, h:h + 1])
        ssum = ptiny.tile([128, 1], F32, tag="tiny", name="ssum")
        nc.vector.tensor_scalar_add(ssum[:], prsums[:, h:h + 1], 1e-8)
        nc.vector.reciprocal(ssum[:], ssum[:])
        nc.vector.tensor_scalar_mul(pr[:], pr[:], ssum[:])
        nc.sync.dma_start(prior_d[h], pr[:])

    # ---- projections: QT/KT in [dout, l] layout (l-half 0 first so the
    # first scores can start ASAP); V emitted later, inside block 0 ----
    QT = [cpool.tile([128, L], F32R, tag=f"QT{dc}", name=f"QT{dc}_{_rep}")
          for dc in range(2)]
    KT = [cpool.tile([128, L], F32R, tag=f"KT{dc}", name=f"KT{dc}_{_rep}")
          for dc in range(2)]
    for lh in range(2):
        for wk, dst in (("k", KT), ("q", QT)):
            for dc in range(2):
                ps = pmm2.tile([128, 512], F32, tag="mm2", name="ps")
                for kc in range(2):
                    nc.tensor.matmul(
                        ps[:],
                        w_sb[wk][:, kc, dc * 128:(dc + 1) * 128],
                        xT[kc][:, lh * 512:(lh + 1) * 512],
                        start=(kc == 0), stop=False)
                nc.tensor.matmul(
                    ps[:],
                    b_sb[wk][0:1, dc * 128:(dc + 1) * 128],
                    ones_r[0:1, lh * 512:(lh + 1) * 512],
                    start=False, stop=True)
                nc.vector.tensor_copy(
                    dst[dc][:, lh * 512:(lh + 1) * 512], ps[:])
    V_sb = cpool.tile([128, 8, D], F32R, tag="V_sb", name=f"V_sb_{_rep}")

    # ---- main loop: per l-chunk li emit the [l,s] pipeline for all
    # heads plus head h=li's [s,l] pipeline (scoresT -> expT -> A@V) ----
    sums = cpool.tile([128, 8, H], F32, tag="sums", name=f"sums_{_rep}")
    wUTraw = [cpool.tile([128, L], F32R, tag=f"wUTraw{g}",
                         name=f"wUTraw{g}_{_rep}") for g in range(2)]
    for li in range(8):
        ebig = pattn.tile([128, H, L], F32, tag="attn", bufs=2, name="ebig")
        for h in range(H):
            dc, hp = h // 4, h % 4
            pss = pmm2.tile([128, L], F32, tag="mm2", name="pss")
            for sh in range(2):
                nc.tensor.matmul(
                    pss[:, sh * 512:(sh + 1) * 512],
                    QT[dc][hp * 32:(hp + 1) * 32, li * 128:(li + 1) * 128],
                    KT[dc][hp * 32:(hp + 1) * 32, sh * 512:(sh + 1) * 512],
                    start=True, stop=True, tile_position=(hp * 32, 0))
            nc.scalar.activation(ebig[:, h, :], pss[:], AF.Exp, scale=SCALE,
                                 accum_out=sums[:, li, h:h + 1])
            inv1 = ptiny.tile([128, 1], F32, tag="tiny", name="inv1")
            nc.vector.reciprocal(inv1[:], sums[:, li, h:h + 1])
            nc.vector.tensor_scalar_mul(ebig[:, h, :], ebig[:, h, :],
                                        inv1[:])
        nc.sync.dma_start(
            attn_d[:, li * 128:(li + 1) * 128, :].rearrange(
                "h l s -> l h s"), ebig[:])

        if li == 0:
            # V projection: PE fills it while ACT runs block 0's exps
            for si in range(8):
                ps = pmm2.tile([128, D], F32, tag="mm2", name="ps")
                for kc in range(2):
                    nc.tensor.matmul(
                        ps[:], xT[kc][:, si * 128:(si + 1) * 128],
                        w_sb["v"][:, kc, :], start=(kc == 0), stop=False)
                nc.tensor.matmul(ps[:], ones_r[0:1, 0:128],
                                 b_sb["v"][0:1, :], start=False, stop=True)
                nc.vector.tensor_copy(V_sb[:, si, :], ps[:])

        h = li
        g, hp = h // 4, h % 4
        eTs = []
        for si in range(8):
            pst = pmm2.tile([128, L], F32, tag="mm2", name="pst")
            for lh in range(2):
                nc.tensor.matmul(
                    pst[:, lh * 512:(lh + 1) * 512],
                    KT[g][hp * 32:(hp + 1) * 32, si * 128:(si + 1) * 128],
                    QT[g][hp * 32:(hp + 1) * 32, lh * 512:(lh + 1) * 512],
                    start=True, stop=True, tile_position=(hp * 32, 0))
            eT = peT.tile([128, L], F32R, tag="eT", name="eT")
            nc.scalar.activation(eT[:], pst[:], AF.Exp, scale=SCALE)
            eTs.append(eT)
        pvt = pmm2.tile([32, L], F32, tag="mm2", name="pvt")
        for lh in range(2):
            for si in range(8):
                nc.tensor.matmul(
                    pvt[:, lh * 512:(lh + 1) * 512],
                    V_sb[:, si, h * 32:(h + 1) * 32],
                    eTs[si][:, lh * 512:(lh + 1) * 512],
                    start=(si == 0), stop=(si == 7))
        nc.vector.tensor_copy(wUTraw[g][hp * 32:(hp + 1) * 32, :], pvt[:])

    # ---- invSrows: 1/S[h, l] as rows ----
    ps_sr = pmm2.tile([8, L], F32, tag="mm2", name="ps_sr")
    for li in range(8):
        nc.tensor.transpose(ps_sr[0:8, li * 128:(li + 1) * 128],
                            sums[:, li, :], eye_sb[:])
    invSrows = cpool.tile([8, L], F32, tag="invSrows", name=f"invSrows_{_rep}")
    nc.vector.reciprocal(invSrows[:], ps_sr[0:8, :])
    invSrows_r = cpool.tile([8, L], F32R, tag="invSrows_r", name=f"invSrows_r_{_rep}")
    nc.vector.tensor_copy(invSrows_r[:], invSrows[:])

    # ---- wUT = wUTraw * broadcast(1/S) in place (f32r on write);
    # broadcast via selection-matrix matmul (K=8), read from PSUM ----
    wUT = [wUTraw[g][:] for g in range(2)]
    for g in range(2):
        psbc = pmm2.tile([128, L], F32, tag="mm2", name="psbc")
        for lh in range(2):
            nc.tensor.matmul(
                psbc[:, lh * 512:(lh + 1) * 512],
                bsel_sb[:, g, :],
                invSrows_r[:, lh * 512:(lh + 1) * 512],
                start=True, stop=True)
        for lh in range(2):
            nc.vector.tensor_tensor(
                wUT[g][:, lh * 512:(lh + 1) * 512],
                wUTraw[g][:, lh * 512:(lh + 1) * 512],
                psbc[:, lh * 512:(lh + 1) * 512], op=ALU.mult)

    # ---- out = weighted @ Wo + bo, transpose to natural, one DMA ----
    outT = [cpool.tile([128, L], F32, tag=f"outT{go}",
                       name=f"outT{go}_{_rep}") for go in range(2)]
    for go in range(2):
        for lh in range(2):
            ps = pmm2.tile([128, 512], F32, tag="mm2", name="ps")
            for gi in range(2):
                nc.tensor.matmul(
                    ps[:], w_sb["o"][:, gi, go * 128:(go + 1) * 128],
                    wUT[gi][:, lh * 512:(lh + 1) * 512],
                    start=(gi == 0), stop=False)
            nc.tensor.matmul(
                ps[:], b_sb["o"][0:1, go * 128:(go + 1) * 128],
                ones_r[0:1, lh * 512:(lh + 1) * 512],
                start=False, stop=True)
            nc.scalar.copy(outT[go][:, lh * 512:(lh + 1) * 512], ps[:])
    obig = pattn.tile([128, 8, D], F32, tag="obig", bufs=1, name="obig")
    for li in range(8):
        pso = pmm2.tile([128, D], F32, tag="mm2", name="pso")
        for go in range(2):
            nc.tensor.transpose(
                pso[:, go * 128:(go + 1) * 128],
                outT[go][:, li * 128:(li + 1) * 128], eye_sb[:])
        nc.scalar.copy(obig[:, li, :], pso[:])
        if li == 3:
            nc.sync.dma_start(
                out_d.rearrange("(a p) d -> p a d", p=128)[:, 0:4, :],
                obig[:, 0:4, :])
    nc.sync.dma_start(out_d.rearrange("(a p) d -> p a d", p=128)[:, 4:8, :],
                      obig[:, 4:8, :])


def _get_nc():
    if "nc" not in _CACHE:
        _CACHE["nc"] = _build()
    return _CACHE["nc"]


def make_in_maps(inputs):
    x = np.asarray(inputs["x"], dtype=np.float32)
    ws = {k: np.ascontiguousarray(np.asarray(inputs[f"W{k}"], np.float32))
          for k in "qkvo"}
    bs = {k: np.ascontiguousarray(
        np.asarray(inputs[f"b{k}"], np.float32).reshape(1, D))
        for k in "qkvo"}
    sig = np.asarray(inputs["prior_sigma"], np.float32).reshape(1, H)
    eye = np.eye(128, dtype=np.float32)
    ones = np.ones((1, L), dtype=np.float32)
    bsel = np.zeros((8, 2, 128), dtype=np.float32)
    for g in range(2):
        for h in range(8):
            for p in range(128):
                if h == g * 4 + p // 32:
                    bsel[h, g, p] = 1.0
    pos = np.arange(L, dtype=np.float64)

    in_maps = []
    for c in range(N_CORES):
        rows = pos[c * 128:(c + 1) * 128]
        dist2n = (-((rows[:, None] - pos[None, :]) ** 2)).astype(np.float32)
        xTc = np.ascontiguousarray(x[c].T).reshape(2, 128, L)
        m = {"xT": xTc, "sig": sig, "dist2n": dist2n,
             "eye": eye, "ones": ones, "bsel": bsel}
        for k in "qkvo":
            m[f"W{k}"] = ws[k]
            m[f"b{k}"] = bs[k]
        in_maps.append(m)
    return in_maps


def kernel(**inputs):
    from concourse.bass_utils import run_bass_kernel_spmd

    nc = _get_nc()
    in_maps = make_in_maps(inputs)
    res = run_bass_kernel_spmd(nc, in_maps, list(range(N_CORES)))
    _CACHE["last_results"] = res

    out = np.stack([res.results[c]["out"] for c in range(N_CORES)])
    series = np.stack([res.results[c]["attn"] for c in range(N_CORES)])
    prior_h = np.concatenate(
        [res.results[c]["prior"] for c in range(N_CORES)], axis=1)
    prior = np.broadcast_to(prior_h[None], (B, H, L, L))
    return (out, series, prior)


# revision 36
# speedup vs baseline: 1.4372x; 1.0229x over previous
"""Trainium2 Bass kernel for the AnomalyBlock problem.

Strategy: data-parallel over batch B=8 (one batch element per NeuronCore).
Each core runs the full attention block for its batch element:
  - q/k/v projections as float32r matmuls against a host-pre-transposed
    xT; biases folded in as K=1 rank-1 matmuls against a ones-row.
  - scores computed in BOTH orientations: [l,s] for the normalized
    attention-map output (ACT exp with accum_out giving row sums for
    free, DVE per-partition normalize), and [s,l] for the A@V
    contraction (the PE contracts over the partition dim, so the moving
    operand must carry s on partitions).
  - A@V accumulates per head into one PSUM tile; results are copied raw
    to SBUF and the 1/S normalization is applied at the end via a
    selection-matrix matmul broadcast (invS varies per (head, l)).
  - the [l,s] pipeline and the [s,l] pipeline are interleaved per
    l-chunk so DMA (attention-map writes) and ACT (exp) overlap.
prior_attn is batch-broadcast, so only [H,L,L] is unique; its rows are
sharded across the 8 cores (core c computes rows c*128..c*128+127 for
all heads) and the batch dim is broadcast on the host at gather time.
"""
import sys
import numpy as np

sys.path.insert(0, "/opt/trn_rl_repo")

B, L, D, H, DK = 8, 1024, 256, 8, 32
N_CORES = 8
SCALE = 1.0 / np.sqrt(DK)

_CACHE = {}


def _build(repeat=1):
    import concourse.bacc as bacc
    import concourse.mybir as mybir
    import concourse.tile as tile

    F32 = mybir.dt.float32
    F32R = mybir.dt.float32r
    AF = mybir.ActivationFunctionType
    ALU = mybir.AluOpType

    nc = bacc.Bacc("TRN2", target_bir_lowering=False, debug=False,
                   num_devices=N_CORES)

    xT_d = nc.dram_tensor("xT", [2, 128, L], F32, kind="ExternalInput")
    w_d = {k: nc.dram_tensor(f"W{k}", [D, D], F32, kind="ExternalInput")
           for k in "qkvo"}
    b_d = {k: nc.dram_tensor(f"b{k}", [1, D], F32, kind="ExternalInput")
           for k in "qkvo"}
    sig_d = nc.dram_tensor("sig", [1, H], F32, kind="ExternalInput")
    dist2n_d = nc.dram_tensor("dist2n", [128, L], F32, kind="ExternalInput")
    eye_d = nc.dram_tensor("eye", [128, 128], F32, kind="ExternalInput")
    ones_d = nc.dram_tensor("ones", [1, L], F32, kind="ExternalInput")
    bsel_d = nc.dram_tensor("bsel", [8, 2, 128], F32, kind="ExternalInput")

    attn_d = nc.dram_tensor("attn", [H, L, L], F32, kind="ExternalOutput")
    prior_d = nc.dram_tensor("prior", [H, 128, L], F32, kind="ExternalOutput")
    out_d = nc.dram_tensor("out", [L, D], F32, kind="ExternalOutput")

    with tile.TileContext(nc) as tc:
        with (
            tc.tile_pool(name="const", bufs=1) as cpool,
            tc.tile_pool(name="attnp", bufs=2) as pattn,
            tc.tile_pool(name="eTp", bufs=8) as peT,
            tc.tile_pool(name="tinyp", bufs=8) as ptiny,
            tc.tile_pool(name="mm2", bufs=4, space="PSUM") as pmm2,
        ):
            # ---- input DMAs: small ones first so they aren't queued
            # behind the big loads; f32r tensors staged f32 + DVE cast ----
            sig_sb = cpool.tile([1, H], F32)
            nc.sync.dma_start(sig_sb[:], sig_d[:])
            dist2n_sb = cpool.tile([128, L], F32)
            nc.sync.dma_start(dist2n_sb[:], dist2n_d[:])
            eye_sb = cpool.tile([128, 128], F32)
            nc.sync.dma_start(eye_sb[:], eye_d[:])
            ones_f = cpool.tile([1, 128], F32)
            nc.sync.dma_start(ones_f[:], ones_d[0:1, 0:128])

            # sigma -> 1/(2 sigma^2) -> per-partition scales (early: the
            # prior exps fill ACT while the projections build)
            t0 = cpool.tile([1, H], F32, tag="t0")
            nc.scalar.activation(t0[:], sig_sb[:], AF.Abs)
            nc.vector.tensor_scalar_add(t0[:], t0[:], 1e-6)
            nc.vector.tensor_tensor(t0[:], t0[:], t0[:], op=ALU.mult)
            nc.vector.tensor_scalar_mul(t0[:], t0[:], 2.0)
            inv2s = cpool.tile([1, H], F32, tag="inv2s")
            nc.vector.reciprocal(inv2s[:], t0[:])
            ps_sc = pmm2.tile([128, H], F32, tag="mm2", name="ps_sc")
            nc.tensor.matmul(ps_sc[:], ones_f[0:1, 0:128], inv2s[:],
                             start=True, stop=True)
            scales = cpool.tile([128, H], F32)
            nc.vector.tensor_copy(scales[:], ps_sc[:])

            ones_r = cpool.tile([1, L], F32R)
            stg1 = pattn.tile([128, L], F32, tag="pr", bufs=2, name="stg1")
            nc.sync.dma_start(stg1[0:1, :], ones_d[:])
            nc.vector.tensor_copy(ones_r[:], stg1[0:1, :])
            xT = [cpool.tile([128, L], F32R, tag=f"xT{dc}", name=f"xT{dc}")
                  for dc in range(2)]
            for dc in range(2):
                stg = pattn.tile([128, L], F32, tag="pr", bufs=2, name="stg")
                nc.sync.dma_start(stg[:], xT_d[dc])
                nc.vector.tensor_copy(xT[dc][:], stg[:])
            w_sb = {}
            b_sb = {}
            for k in "qk":
                w_sb[k] = cpool.tile([128, 2, D], F32R, tag=f"W{k}",
                                     name=f"W{k}_sb")
                stgw = pattn.tile([128, 2, D], F32, tag="stgw", bufs=2,
                                  name="stgw")
                nc.sync.dma_start(
                    stgw[:], w_d[k].rearrange("(k p) n -> p k n", p=128))
                nc.vector.tensor_copy(w_sb[k][:], stgw[:])
                b_sb[k] = cpool.tile([1, D], F32R, tag=f"b{k}",
                                     name=f"b{k}_sb")
                stgb = pattn.tile([1, D], F32, tag="stgb", bufs=2,
                                  name="stgb")
                nc.sync.dma_start(stgb[:], b_d[k][:])
                nc.vector.tensor_copy(b_sb[k][:], stgb[:])

            for _rep in range(repeat):
                _emit_compute(nc, tc, cpool, pattn, peT, ptiny, pmm2,
                              F32, F32R, AF, ALU, xT, w_sb, b_sb, scales,
                              dist2n_sb, eye_sb, ones_f, ones_r,
                              w_d, b_d, bsel_d,
                              attn_d, prior_d, out_d, _rep)

    nc.compile()
    return nc


def _emit_compute(nc, tc, cpool, pattn, peT, ptiny, pmm2,
                  F32, F32R, AF, ALU, xT, w_sb, b_sb, scales, dist2n_sb,
                  eye_sb, ones_f, ones_r, w_d, b_d, bsel_d,
                  attn_d, prior_d, out_d, _rep):
    _late = {}
    # ---- prior tiles: exp(dist2n * scale_h), row-normalized ----
    prsums = cpool.tile([128, H], F32, tag="prsums", name=f"prsums_{_rep}")
    for h in range(H):
        pr = pattn.tile([128, L], F32, tag="pr", bufs=2, name="pr")
        nc.scalar.activation(pr[:], dist2n_sb[:], AF.Exp,
                             scale=scales[:, h:h + 1],
                             accum_out=prsums[:, h:h + 1])
        ssum = ptiny.tile([128, 1], F32, tag="tiny", name="ssum")
        nc.vector.tensor_scalar_add(ssum[:], prsums[:, h:h + 1], 1e-8)
        nc.vector.reciprocal(ssum[:], ssum[:])
        nc.vector.tensor_scalar_mul(pr[:], pr[:], ssum[:])
        nc.sync.dma_start(prior_d[h], pr[:])

    # ---- projections: QT/KT in [dout, l] layout (l-half 0 first so the
    # first scores can start ASAP); V emitted later, inside block 0 ----
    QT = [cpool.tile([128, L], F32R, tag=f"QT{dc}", name=f"QT{dc}_{_rep}")
          for dc in range(2)]
    KT = [cpool.tile([128, L], F32R, tag=f"KT{dc}", name=f"KT{dc}_{_rep}")
          for dc in range(2)]
    for lh in range(2):
        for wk, dst in (("k", KT), ("q", QT)):
            for dc in range(2):
                ps = pmm2.tile([128, 512], F32, tag="mm2", name="ps")
                for kc in range(2):
                    nc.tensor.matmul(
                        ps[:],
                        w_sb[wk][:, kc, dc * 128:(dc + 1) * 128],
                        xT[kc][:, lh * 512:(lh + 1) * 512],
                        start=(kc == 0), stop=False)
                nc.tensor.matmul(
                    ps[:],
                    b_sb[wk][0:1, dc * 128:(dc + 1) * 128],
                    ones_r[0:1, lh * 512:(lh + 1) * 512],
                    start=False, stop=True)
                nc.vector.tensor_copy(
                    dst[dc][:, lh * 512:(lh + 1) * 512], ps[:])
    V_sb = cpool.tile([128, 8, D], F32R, tag="V_sb", name=f"V_sb_{_rep}")

    # ---- main loop: per l-chunk li emit the [l,s] pipeline for all
    # heads plus head h=li's [s,l] pipeline (scoresT -> expT -> A@V) ----
    sums = cpool.tile([128, 8, H], F32, tag="sums", name=f"sums_{_rep}")
    wUTraw = [cpool.tile([128, L], F32R, tag=f"wUTraw{g}",
                         name=f"wUTraw{g}_{_rep}") for g in range(2)]
    for li in range(8):
        ebig = pattn.tile([128, H, L], F32, tag="attn", bufs=2, name="ebig")
        for h in range(H):
            dc, hp = h // 4, h % 4
            pss = pmm2.tile([128, L], F32, tag="mm2", name="pss")
            for sh in range(2):
                nc.tensor.matmul(
                    pss[:, sh * 512:(sh + 1) * 512],
                    QT[dc][hp * 32:(hp + 1) * 32, li * 128:(li + 1) * 128],
                    KT[dc][hp * 32:(hp + 1) * 32, sh * 512:(sh + 1) * 512],
                    start=True, stop=True, tile_position=(hp * 32, 0))
            nc.scalar.activation(ebig[:, h, :], pss[:], AF.Exp, scale=SCALE,
                                 accum_out=sums[:, li, h:h + 1])
            inv1 = ptiny.tile([128, 1], F32, tag="tiny", name="inv1")
            nc.vector.reciprocal(inv1[:], sums[:, li, h:h + 1])
            nc.vector.tensor_scalar_mul(ebig[:, h, :], ebig[:, h, :],
                                        inv1[:])
        nc.sync.dma_start(
            attn_d[:, li * 128:(li + 1) * 128, :].rearrange(
                "h l s -> l h s"), ebig[:])

        if li == 0:
            # V projection: PE fills it while ACT runs block 0's exps
            for k in "v":
                w_sb[k] = cpool.tile([128, 2, D], F32R, tag=f"W{k}",
                                     name=f"W{k}_sb_{_rep}")
                stgw = pattn.tile([128, 2, D], F32, tag="stgw", bufs=2,
                                  name="stgw")
                nc.sync.dma_start(
                    stgw[:], w_d[k].rearrange("(k p) n -> p k n", p=128))
                nc.vector.tensor_copy(w_sb[k][:], stgw[:])
                b_sb[k] = cpool.tile([1, D], F32R, tag=f"b{k}",
                                     name=f"b{k}_sb_{_rep}")
                stgb = pattn.tile([1, D], F32, tag="stgb", bufs=2,
                                  name="stgb")
                nc.sync.dma_start(stgb[:], b_d[k][:])
                nc.vector.tensor_copy(b_sb[k][:], stgb[:])
            for si in range(8):
                ps = pmm2.tile([128, D], F32, tag="mm2", name="ps")
                for kc in range(2):
                    nc.tensor.matmul(
                        ps[:], xT[kc][:, si * 128:(si + 1) * 128],
                        w_sb["v"][:, kc, :], start=(kc == 0), stop=False)
                nc.tensor.matmul(ps[:], ones_r[0:1, 0:128],
                                 b_sb["v"][0:1, :], start=False, stop=True)
                nc.vector.tensor_copy(V_sb[:, si, :], ps[:])

        if li == 5:
            for k in "o":
                w_sb[k] = cpool.tile([128, 2, D], F32R, tag=f"W{k}",
                                     name=f"W{k}_sb_{_rep}")
                stgw = pattn.tile([128, 2, D], F32, tag="stgw", bufs=2,
                                  name="stgw")
                nc.sync.dma_start(
                    stgw[:], w_d[k].rearrange("(k p) n -> p k n", p=128))
                nc.vector.tensor_copy(w_sb[k][:], stgw[:])
                b_sb[k] = cpool.tile([1, D], F32R, tag=f"b{k}",
                                     name=f"b{k}_sb_{_rep}")
                stgb = pattn.tile([1, D], F32, tag="stgb", bufs=2,
                                  name="stgb")
                nc.sync.dma_start(stgb[:], b_d[k][:])
                nc.vector.tensor_copy(b_sb[k][:], stgb[:])
            bsel_sb = cpool.tile([8, 2, 128], F32R, tag="bsel",
                                 name=f"bsel_sb_{_rep}")
            stgs = pattn.tile([8, 2, 128], F32, tag="stgb", bufs=2,
                              name="stgs")
            nc.sync.dma_start(stgs[:], bsel_d[:])
            nc.vector.tensor_copy(bsel_sb[:], stgs[:])
            _late["bsel_sb"] = bsel_sb

        h = li
        g, hp = h // 4, h % 4
        eTs = []
        for si in range(8):
            pst = pmm2.tile([128, L], F32, tag="mm2", name="pst")
            for lh in range(2):
                nc.tensor.matmul(
                    pst[:, lh * 512:(lh + 1) * 512],
                    KT[g][hp * 32:(hp + 1) * 32, si * 128:(si + 1) * 128],
                    QT[g][hp * 32:(hp + 1) * 32, lh * 512:(lh + 1) * 512],
                    start=True, stop=True, tile_position=(hp * 32, 0))
            eT = peT.tile([128, L], F32R, tag="eT", name="eT")
            nc.scalar.activation(eT[:], pst[:], AF.Exp, scale=SCALE)
            eTs.append(eT)
        pvt = pmm2.tile([32, L], F32, tag="mm2", name="pvt")
        for lh in range(2):
            for si in range(8):
                nc.tensor.matmul(
                    pvt[:, lh * 512:(lh + 1) * 512],
                    V_sb[:, si, h * 32:(h + 1) * 32],
                    eTs[si][:, lh * 512:(lh + 1) * 512],
                    start=(si == 0), stop=(si == 7))
        nc.vector.tensor_copy(wUTraw[g][hp * 32:(hp + 1) * 32, :], pvt[:])

    # ---- invSrows: 1/S[h, l] as rows ----
    ps_sr = pmm2.tile([8, L], F32, tag="mm2", name="ps_sr")
    for li in range(8):
        nc.tensor.transpose(ps_sr[0:8, li * 128:(li + 1) * 128],
                            sums[:, li, :], eye_sb[:])
    invSrows = cpool.tile([8, L], F32, tag="invSrows", name=f"invSrows_{_rep}")
    nc.vector.reciprocal(invSrows[:], ps_sr[0:8, :])
    invSrows_r = cpool.tile([8, L], F32R, tag="invSrows_r", name=f"invSrows_r_{_rep}")
    nc.vector.tensor_copy(invSrows_r[:], invSrows[:])

    # ---- wUT = wUTraw * broadcast(1/S) in place (f32r on write);
    # broadcast via selection-matrix matmul (K=8), read from PSUM ----
    wUT = [wUTraw[g][:] for g in range(2)]
    for g in range(2):
        psbc = pmm2.tile([128, L], F32, tag="mm2", name="psbc")
        for lh in range(2):
            nc.tensor.matmul(
                psbc[:, lh * 512:(lh + 1) * 512],
                _late["bsel_sb"][:, g, :],
                invSrows_r[:, lh * 512:(lh + 1) * 512],
                start=True, stop=True)
        for lh in range(2):
            nc.vector.tensor_tensor(
                wUT[g][:, lh * 512:(lh + 1) * 512],
                wUTraw[g][:, lh * 512:(lh + 1) * 512],
                psbc[:, lh * 512:(lh + 1) * 512], op=ALU.mult)

    # ---- out = weighted @ Wo + bo, transpose to natural, one DMA ----
    outT = [cpool.tile([128, L], F32, tag=f"outT{go}",
                       name=f"outT{go}_{_rep}") for go in range(2)]
    for go in range(2):
        for lh in range(2):
            ps = pmm2.tile([128, 512], F32, tag="mm2", name="ps")
            for gi in range(2):
                nc.tensor.matmul(
                    ps[:], w_sb["o"][:, gi, go * 128:(go + 1) * 128],
                    wUT[gi][:, lh * 512:(lh + 1) * 512],
                    start=(gi == 0), stop=False)
            nc.tensor.matmul(
                ps[:], b_sb["o"][0:1, go * 128:(go + 1) * 128],
                ones_r[0:1, lh * 512:(lh + 1) * 512],
                start=False, stop=True)
            nc.scalar.copy(outT[go][:, lh * 512:(lh + 1) * 512], ps[:])
    obig = pattn.tile([128, 8, D], F32, tag="obig", bufs=1, name="obig")
    for li in range(8):
        pso = pmm2.tile([128, D], F32, tag="mm2", name="pso")
        for go in range(2):
            nc.tensor.transpose(
                pso[:, go * 128:(go + 1) * 128],
                outT[go][:, li * 128:(li + 1) * 128], eye_sb[:])
        nc.scalar.copy(obig[:, li, :], pso[:])
        if li == 3:
            nc.sync.dma_start(
                out_d.rearrange("(a p) d -> p a d", p=128)[:, 0:4, :],
                obig[:, 0:4, :])
    nc.sync.dma_start(out_d.rearrange("(a p) d -> p a d", p=128)[:, 4:8, :],
                      obig[:, 4:8, :])


def _get_nc():
    if "nc" not in _CACHE:
        _CACHE["nc"] = _build()
    return _CACHE["nc"]


def make_in_maps(inputs):
    x = np.asarray(inputs["x"], dtype=np.float32)
    ws = {k: np.ascontiguousarray(np.asarray(inputs[f"W{k}"], np.float32))
          for k in "qkvo"}
    bs = {k: np.ascontiguousarray(
        np.asarray(inputs[f"b{k}"], np.float32).reshape(1, D))
        for k in "qkvo"}
    sig = np.asarray(inputs["prior_sigma"], np.float32).reshape(1, H)
    eye = np.eye(128, dtype=np.float32)
    ones = np.ones((1, L), dtype=np.float32)
    bsel = np.zeros((8, 2, 128), dtype=np.float32)
    for g in range(2):
        for h in range(8):
            for p in range(128):
                if h == g * 4 + p // 32:
                    bsel[h, g, p] = 1.0
    pos = np.arange(L, dtype=np.float64)

    in_maps = []
    for c in range(N_CORES):
        rows = pos[c * 128:(c + 1) * 128]
        dist2n = (-((rows[:, None] - pos[None, :]) ** 2)).astype(np.float32)
        xTc = np.ascontiguousarray(x[c].T).reshape(2, 128, L)
        m = {"xT": xTc, "sig": sig, "dist2n": dist2n,
             "eye": eye, "ones": ones, "bsel": bsel}
        for k in "qkvo":
            m[f"W{k}"] = ws[k]
            m[f"b{k}"] = bs[k]
        in_maps.append(m)
    return in_maps


def kernel(**inputs):
    from concourse.bass_utils import run_bass_kernel_spmd

    nc = _get_nc()
    in_maps = make_in_maps(inputs)
    res = run_bass_kernel_spmd(nc, in_maps, list(range(N_CORES)))
    _CACHE["last_results"] = res

    out = np.stack([res.results[c]["out"] for c in range(N_CORES)])
    series = np.stack([res.results[c]["attn"] for c in range(N_CORES)])
    prior_h = np.concatenate(
        [res.results[c]["prior"] for c in range(N_CORES)], axis=1)
    prior = np.broadcast_to(prior_h[None], (B, H, L, L))
    return (out, series, prior)


# revision 41
# speedup vs baseline: 2.8508x; 1.9836x over previous
"""Trainium2 Bass kernel for the AnomalyBlock problem.

Strategy: data-parallel over batch B=8 (one batch element per NeuronCore).
Each core runs the full attention block for its batch element:
  - q/k/v projections as float32r matmuls against a host-pre-transposed
    xT; biases folded in as K=1 rank-1 matmuls against a ones-row.
  - scores computed in BOTH orientations: [l,s] for the normalized
    attention-map output (ACT exp with accum_out giving row sums for
    free, DVE per-partition normalize), and [s,l] for the A@V
    contraction (the PE contracts over the partition dim, so the moving
    operand must carry s on partitions).
  - A@V accumulates per head into one PSUM tile; results are copied raw
    to SBUF and the 1/S normalization is applied at the end via a
    selection-matrix matmul broadcast (invS varies per (head, l)).
  - the [l,s] pipeline and the [s,l] pipeline are interleaved per
    l-chunk so DMA (attention-map writes) and ACT (exp) overlap.
prior_attn is batch-broadcast, so only [H,L,L] is unique; its rows are
sharded across the 8 cores (core c computes rows c*128..c*128+127 for
all heads) and the batch dim is broadcast on the host at gather time.
"""
import sys
import numpy as np

sys.path.insert(0, "/opt/trn_rl_repo")

B, L, D, H, DK = 8, 1024, 256, 8, 32
N_CORES = 8
SCALE = 1.0 / np.sqrt(DK)

_CACHE = {}


def _build(repeat=1):
    import concourse.bacc as bacc
    import concourse.mybir as mybir
    import concourse.tile as tile

    F32 = mybir.dt.float32
    F32R = mybir.dt.float32r
    AF = mybir.ActivationFunctionType
    ALU = mybir.AluOpType

    nc = bacc.Bacc("TRN2", target_bir_lowering=False, debug=False,
                   num_devices=N_CORES)

    xT_d = nc.dram_tensor("xT", [2, 128, L], F32, kind="ExternalInput")
    w_d = {k: nc.dram_tensor(f"W{k}", [D, D], F32, kind="ExternalInput")
           for k in "qkvo"}
    b_d = {k: nc.dram_tensor(f"b{k}", [1, D], F32, kind="ExternalInput")
           for k in "qkvo"}
    sig_d = nc.dram_tensor("sig", [1, H], F32, kind="ExternalInput")
    dist2n_d = nc.dram_tensor("dist2n", [128, L], F32, kind="ExternalInput")
    eye_d = nc.dram_tensor("eye", [128, 128], F32, kind="ExternalInput")
    ones_d = nc.dram_tensor("ones", [1, L], F32, kind="ExternalInput")
    bsel_d = nc.dram_tensor("bsel", [8, 2, 128], F32, kind="ExternalInput")

    attn_d = nc.dram_tensor("attn", [H, L, L], F32, kind="ExternalOutput")
    prior_d = nc.dram_tensor("prior", [H, 128, L], F32, kind="ExternalOutput")
    out_d = nc.dram_tensor("out", [L, D], F32, kind="ExternalOutput")

    with tile.TileContext(nc) as tc:
        with (
            tc.tile_pool(name="const", bufs=1) as cpool,
            tc.tile_pool(name="attnp", bufs=2) as pattn,
            tc.tile_pool(name="eTp", bufs=8) as peT,
            tc.tile_pool(name="tinyp", bufs=8) as ptiny,
            tc.tile_pool(name="mm2", bufs=4, space="PSUM") as pmm2,
        ):
            # ---- input DMAs: small ones first so they aren't queued
            # behind the big loads; f32r tensors staged f32 + DVE cast ----
            sig_sb = cpool.tile([1, H], F32)
            nc.sync.dma_start(sig_sb[:], sig_d[:])
            dist2n_sb = cpool.tile([128, L], F32)
            nc.sync.dma_start(dist2n_sb[:], dist2n_d[:])
            eye_sb = cpool.tile([128, 128], F32)
            nc.sync.dma_start(eye_sb[:], eye_d[:])
            ones_f = cpool.tile([1, 128], F32)
            nc.sync.dma_start(ones_f[:], ones_d[0:1, 0:128])

            # sigma -> 1/(2 sigma^2) -> per-partition scales (early: the
            # prior exps fill ACT while the projections build)
            t0 = cpool.tile([1, H], F32, tag="t0")
            nc.scalar.activation(t0[:], sig_sb[:], AF.Abs)
            nc.vector.tensor_scalar_add(t0[:], t0[:], 1e-6)
            nc.vector.tensor_tensor(t0[:], t0[:], t0[:], op=ALU.mult)
            nc.vector.tensor_scalar_mul(t0[:], t0[:], 2.0)
            inv2s = cpool.tile([1, H], F32, tag="inv2s")
            nc.vector.reciprocal(inv2s[:], t0[:])
            ps_sc = pmm2.tile([128, H], F32, tag="mm2", name="ps_sc")
            nc.tensor.matmul(ps_sc[:], ones_f[0:1, 0:128], inv2s[:],
                             start=True, stop=True)
            scales = cpool.tile([128, H], F32)
            nc.vector.tensor_copy(scales[:], ps_sc[:])

            ones_r = cpool.tile([1, L], F32R)
            stg1 = pattn.tile([128, L], F32, tag="pr", bufs=3, name="stg1")
            nc.sync.dma_start(stg1[0:1, :], ones_d[:])
            nc.vector.tensor_copy(ones_r[:], stg1[0:1, :])
            xT = [cpool.tile([128, L], F32R, tag=f"xT{dc}", name=f"xT{dc}")
                  for dc in range(2)]
            for dc in range(2):
                stg = pattn.tile([128, L], F32, tag="pr", bufs=3, name="stg")
                nc.sync.dma_start(stg[:], xT_d[dc])
                nc.vector.tensor_copy(xT[dc][:], stg[:])
            w_sb = {}
            b_sb = {}
            for k in "qk":
                w_sb[k] = cpool.tile([128, 2, D], F32R, tag=f"W{k}",
                                     name=f"W{k}_sb")
                stgw = pattn.tile([128, 2, D], F32, tag="stgw", bufs=2,
                                  name="stgw")
                nc.sync.dma_start(
                    stgw[:], w_d[k].rearrange("(k p) n -> p k n", p=128))
                nc.vector.tensor_copy(w_sb[k][:], stgw[:])
                b_sb[k] = cpool.tile([1, D], F32R, tag=f"b{k}",
                                     name=f"b{k}_sb")
                stgb = pattn.tile([1, D], F32, tag="stgb", bufs=2,
                                  name="stgb")
                nc.sync.dma_start(stgb[:], b_d[k][:])
                nc.vector.tensor_copy(b_sb[k][:], stgb[:])

            for _rep in range(repeat):
                _emit_compute(nc, tc, cpool, pattn, peT, ptiny, pmm2,
                              F32, F32R, AF, ALU, xT, w_sb, b_sb, scales,
                              dist2n_sb, eye_sb, ones_f, ones_r,
                              w_d, b_d, bsel_d,
                              attn_d, prior_d, out_d, _rep)

    nc.compile()
    return nc


def _emit_compute(nc, tc, cpool, pattn, peT, ptiny, pmm2,
                  F32, F32R, AF, ALU, xT, w_sb, b_sb, scales, dist2n_sb,
                  eye_sb, ones_f, ones_r, w_d, b_d, bsel_d,
                  attn_d, prior_d, out_d, _rep):
    _late = {}
    # ---- prior tiles: exp(dist2n * scale_h), row-normalized ----
    prsums = cpool.tile([128, H], F32, tag="prsums", name=f"prsums_{_rep}")
    for h in range(H):
        pr = pattn.tile([128, L], F32, tag="pr", bufs=3, name="pr")
        nc.scalar.activation(pr[:], dist2n_sb[:], AF.Exp,
                             scale=scales[:, h:h + 1],
                             accum_out=prsums[:, h:h + 1])
        ssum = ptiny.tile([128, 1], F32, tag="tiny", name="ssum")
        nc.vector.tensor_scalar_add(ssum[:], prsums[:, h:h + 1], 1e-8)
        nc.vector.reciprocal(ssum[:], ssum[:])
        nc.vector.tensor_scalar_mul(pr[:], pr[:], ssum[:])
        nc.sync.dma_start(prior_d[h], pr[:])

    # ---- projections: QT/KT in [dout, l] layout (l-half 0 first so the
    # first scores can start ASAP); V emitted later, inside block 0 ----
    QT = [cpool.tile([128, L], F32R, tag=f"QT{dc}", name=f"QT{dc}_{_rep}")
          for dc in range(2)]
    KT = [cpool.tile([128, L], F32R, tag=f"KT{dc}", name=f"KT{dc}_{_rep}")
          for dc in range(2)]
    for lh in range(2):
        for wk, dst in (("k", KT), ("q", QT)):
            for dc in range(2):
                ps = pmm2.tile([128, 512], F32, tag="mm2", name="ps")
                for kc in range(2):
                    nc.tensor.matmul(
                        ps[:],
                        w_sb[wk][:, kc, dc * 128:(dc + 1) * 128],
                        xT[kc][:, lh * 512:(lh + 1) * 512],
                        start=(kc == 0), stop=False)
                nc.tensor.matmul(
                    ps[:],
                    b_sb[wk][0:1, dc * 128:(dc + 1) * 128],
                    ones_r[0:1, lh * 512:(lh + 1) * 512],
                    start=False, stop=True)
                nc.vector.tensor_copy(
                    dst[dc][:, lh * 512:(lh + 1) * 512], ps[:])
    V_sb = cpool.tile([128, 8, D], F32R, tag="V_sb", name=f"V_sb_{_rep}")

    # ---- main loop: per l-chunk li emit the [l,s] pipeline for all
    # heads plus head h=li's [s,l] pipeline (scoresT -> expT -> A@V) ----
    sums = cpool.tile([128, 8, H], F32, tag="sums", name=f"sums_{_rep}")
    wUTraw = [cpool.tile([128, L], F32R, tag=f"wUTraw{g}",
                         name=f"wUTraw{g}_{_rep}") for g in range(2)]
    Srows_sb = cpool.tile([8, L], F32, tag="Srows", name=f"Srows_{_rep}")
    for li in range(8):
        ebig = pattn.tile([128, H, L], F32, tag="attn", bufs=2, name="ebig")
        for h in range(H):
            dc, hp = h // 4, h % 4
            pss = pmm2.tile([128, L], F32, tag="mm2", name="pss")
            for sh in range(2):
                nc.tensor.matmul(
                    pss[:, sh * 512:(sh + 1) * 512],
                    QT[dc][hp * 32:(hp + 1) * 32, li * 128:(li + 1) * 128],
                    KT[dc][hp * 32:(hp + 1) * 32, sh * 512:(sh + 1) * 512],
                    start=True, stop=True, tile_position=(hp * 32, 0))
            nc.scalar.activation(ebig[:, h, :], pss[:], AF.Exp, scale=SCALE,
                                 accum_out=sums[:, li, h:h + 1])
            inv1 = ptiny.tile([128, 1], F32, tag="tiny", name="inv1")
            nc.vector.reciprocal(inv1[:], sums[:, li, h:h + 1])
            nc.vector.tensor_scalar_mul(ebig[:, h, :], ebig[:, h, :],
                                        inv1[:])
        nc.sync.dma_start(
            attn_d[:, li * 128:(li + 1) * 128, :].rearrange(
                "h l s -> l h s"), ebig[:])

        if li == 0:
            # V projection: PE fills it while ACT runs block 0's exps
            for k in "v":
                w_sb[k] = cpool.tile([128, 2, D], F32R, tag=f"W{k}",
                                     name=f"W{k}_sb_{_rep}")
                stgw = pattn.tile([128, 2, D], F32, tag="stgw", bufs=2,
                                  name="stgw")
                nc.sync.dma_start(
                    stgw[:], w_d[k].rearrange("(k p) n -> p k n", p=128))
                nc.vector.tensor_copy(w_sb[k][:], stgw[:])
                b_sb[k] = cpool.tile([1, D], F32R, tag=f"b{k}",
                                     name=f"b{k}_sb_{_rep}")
                stgb = pattn.tile([1, D], F32, tag="stgb", bufs=2,
                                  name="stgb")
                nc.sync.dma_start(stgb[:], b_d[k][:])
                nc.vector.tensor_copy(b_sb[k][:], stgb[:])
            for si in range(8):
                ps = pmm2.tile([128, D], F32, tag="mm2", name="ps")
                for kc in range(2):
                    nc.tensor.matmul(
                        ps[:], xT[kc][:, si * 128:(si + 1) * 128],
                        w_sb["v"][:, kc, :], start=(kc == 0), stop=False)
                nc.tensor.matmul(ps[:], ones_r[0:1, 0:128],
                                 b_sb["v"][0:1, :], start=False, stop=True)
                nc.vector.tensor_copy(V_sb[:, si, :], ps[:])

        if li == 5:
            for k in "o":
                w_sb[k] = cpool.tile([128, 2, D], F32R, tag=f"W{k}",
                                     name=f"W{k}_sb_{_rep}")
                stgw = pattn.tile([128, 2, D], F32, tag="stgw", bufs=2,
                                  name="stgw")
                nc.sync.dma_start(
                    stgw[:], w_d[k].rearrange("(k p) n -> p k n", p=128))
                nc.vector.tensor_copy(w_sb[k][:], stgw[:])
                b_sb[k] = cpool.tile([1, D], F32R, tag=f"b{k}",
                                     name=f"b{k}_sb_{_rep}")
                stgb = pattn.tile([1, D], F32, tag="stgb", bufs=2,
                                  name="stgb")
                nc.sync.dma_start(stgb[:], b_d[k][:])
                nc.vector.tensor_copy(b_sb[k][:], stgb[:])
            bsel_sb = cpool.tile([8, 2, 128], F32R, tag="bsel",
                                 name=f"bsel_sb_{_rep}")
            stgs = pattn.tile([8, 2, 128], F32, tag="stgb", bufs=2,
                              name="stgs")
            nc.sync.dma_start(stgs[:], bsel_d[:])
            nc.vector.tensor_copy(bsel_sb[:], stgs[:])
            _late["bsel_sb"] = bsel_sb

        h = li
        g, hp = h // 4, h % 4
        eTs = []
        for si in range(8):
            pst = pmm2.tile([128, L], F32, tag="mm2", name="pst")
            for lh in range(2):
                nc.tensor.matmul(
                    pst[:, lh * 512:(lh + 1) * 512],
                    KT[g][hp * 32:(hp + 1) * 32, si * 128:(si + 1) * 128],
                    QT[g][hp * 32:(hp + 1) * 32, lh * 512:(lh + 1) * 512],
                    start=True, stop=True, tile_position=(hp * 32, 0))
            eT = peT.tile([128, L], F32R, tag="eT", name="eT")
            nc.scalar.activation(eT[:], pst[:], AF.Exp, scale=SCALE)
            eTs.append(eT)
        pvt = pmm2.tile([32, L], F32, tag="mm2", name="pvt")
        for lh in range(2):
            for si in range(8):
                nc.tensor.matmul(
                    pvt[:, lh * 512:(lh + 1) * 512],
                    V_sb[:, si, h * 32:(h + 1) * 32],
                    eTs[si][:, lh * 512:(lh + 1) * 512],
                    start=(si == 0), stop=(si == 7))
        nc.vector.tensor_copy(wUTraw[g][hp * 32:(hp + 1) * 32, :], pvt[:])
        # transpose this block's column of row-sums into Srows
        ps_sr = pmm2.tile([8, 128], F32, tag="mm2", name="ps_sr")
        nc.tensor.transpose(ps_sr[0:8, :], sums[:, li, :], eye_sb[:])
        nc.vector.tensor_copy(Srows_sb[0:8, li * 128:(li + 1) * 128],
                              ps_sr[0:8, :])

    # ---- invSrows: 1/S[h, l] as rows ----
    invSrows = cpool.tile([8, L], F32, tag="invSrows", name=f"invSrows_{_rep}")
    nc.vector.reciprocal(invSrows[:], Srows_sb[0:8, :])
    invSrows_r = cpool.tile([8, L], F32R, tag="invSrows_r", name=f"invSrows_r_{_rep}")
    nc.vector.tensor_copy(invSrows_r[:], invSrows[:])

    # ---- wUT = wUTraw * broadcast(1/S) in place (f32r on write);
    # broadcast via selection-matrix matmul (K=8), read from PSUM ----
    wUT = [wUTraw[g][:] for g in range(2)]
    for g in range(2):
        psbc = pmm2.tile([128, L], F32, tag="mm2", name="psbc")
        for lh in range(2):
            nc.tensor.matmul(
                psbc[:, lh * 512:(lh + 1) * 512],
                _late["bsel_sb"][:, g, :],
                invSrows_r[:, lh * 512:(lh + 1) * 512],
                start=True, stop=True)
        for lh in range(2):
            nc.vector.tensor_tensor(
                wUT[g][:, lh * 512:(lh + 1) * 512],
                wUTraw[g][:, lh * 512:(lh + 1) * 512],
                psbc[:, lh * 512:(lh + 1) * 512], op=ALU.mult)

    # ---- out = weighted @ Wo + bo, transpose to natural, one DMA ----
    outT = [cpool.tile([128, L], F32, tag=f"outT{go}",
                       name=f"outT{go}_{_rep}") for go in range(2)]
    for go in range(2):
        for lh in range(2):
            ps = pmm2.tile([128, 512], F32, tag="mm2", name="ps")
            for gi in range(2):
                nc.tensor.matmul(
                    ps[:], w_sb["o"][:, gi, go * 128:(go + 1) * 128],
                    wUT[gi][:, lh * 512:(lh + 1) * 512],
                    start=(gi == 0), stop=False)
            nc.tensor.matmul(
                ps[:], b_sb["o"][0:1, go * 128:(go + 1) * 128],
                ones_r[0:1, lh * 512:(lh + 1) * 512],
                start=False, stop=True)
            nc.scalar.copy(outT[go][:, lh * 512:(lh + 1) * 512], ps[:])
    obig = pattn.tile([128, 8, D], F32, tag="obig", bufs=1, name="obig")
    for li in range(8):
        pso = pmm2.tile([128, D], F32, tag="mm2", name="pso")
        for go in range(2):
            nc.tensor.transpose(
                pso[:, go * 128:(go + 1) * 128],
                outT[go][:, li * 128:(li + 1) * 128], eye_sb[:])
        nc.scalar.copy(obig[:, li, :], pso[:])
        if li == 3:
            nc.sync.dma_start(
                out_d.rearrange("(a p) d -> p a d", p=128)[:, 0:4, :],
                obig[:, 0:4, :])
    nc.sync.dma_start(out_d.rearrange("(a p) d -> p a d", p=128)[:, 4:8, :],
                      obig[:, 4:8, :])


def _get_nc():
    if "nc" not in _CACHE:
        _CACHE["nc"] = _build()
    return _CACHE["nc"]


def make_in_maps(inputs):
    x = np.asarray(inputs["x"], dtype=np.float32)
    ws = {k: np.ascontiguousarray(np.asarray(inputs[f"W{k}"], np.float32))
          for k in "qkvo"}
    bs = {k: np.ascontiguousarray(
        np.asarray(inputs[f"b{k}"], np.float32).reshape(1, D))
        for k in "qkvo"}
    sig = np.asarray(inputs["prior_sigma"], np.float32).reshape(1, H)
    eye = np.eye(128, dtype=np.float32)
    ones = np.ones((1, L), dtype=np.float32)
    bsel = np.zeros((8, 2, 128), dtype=np.float32)
    for g in range(2):
        for h in range(8):
            for p in range(128):
                if h == g * 4 + p // 32:
                    bsel[h, g, p] = 1.0
    pos = np.arange(L, dtype=np.float64)

    in_maps = []
    for c in range(N_CORES):
        rows = pos[c * 128:(c + 1) * 128]
        dist2n = (-((rows[:, None] - pos[None, :]) ** 2)).astype(np.float32)
        xTc = np.ascontiguousarray(x[c].T).reshape(2, 128, L)
        m = {"xT": xTc, "sig": sig, "dist2n": dist2n,
             "eye": eye, "ones": ones, "bsel": bsel}
        for k in "qkvo":
            m[f"W{k}"] = ws[k]
            m[f"b{k}"] = bs[k]
        in_maps.append(m)
    return in_maps


def kernel(**inputs):
    from concourse.bass_utils import run_bass_kernel_spmd

    nc = _get_nc()
    in_maps = make_in_maps(inputs)
    res = run_bass_kernel_spmd(nc, in_maps, list(range(N_CORES)))
    _CACHE["last_results"] = res

    out = np.stack([res.results[c]["out"] for c in range(N_CORES)])
    series = np.stack([res.results[c]["attn"] for c in range(N_CORES)])
    prior_h = np.concatenate(
        [res.results[c]["prior"] for c in range(N_CORES)], axis=1)
    prior = np.broadcast_to(prior_h[None], (B, H, L, L))
    return (out, series, prior)
